# revision 6
# baseline (speedup 1.0000x reference)
"""Trainium2 Bass kernel for MeanTokenProjectionPool.

Computes, for batch [B,T,D], per-type segmented masked mean over T into G
groups followed by a per-group linear projection (W[g] @ mean + b[g]).

Strategy (data-parallel over B, 4 batch items per core, no cross-core comm):
  - Host precomputes the tiny index tensors: a 0/1 membership mask
    vf[b,t,g] = (token_types[t]==g) & ~pad[b,t], per-(b,g) reciprocal
    counts, and the replicated bias.
  - f32 matmuls on the PE run ~5x slower than bf16 (2 HI/LO passes at half
    stream rate). So all GEMM operands are split hi/lo: x = bf16(x) +
    bf16(x - bf16(x)), giving ~17 mantissa bits (~1e-5 rel err) at bf16
    speed. Same HBM bytes as f32. The mask is 0/1 (exact in bf16).
  - Device phase 1: segment-sums via PE matmul: for each local b,
    sums[8g,512d] += vf_c[128t,8g].T @ hi_c[128t,512d] (+ lo_c) over 32
    token chunks, accumulated in a per-b PSUM bank. One tensor_scalar
    multiply by 1/count -> means[8, 512] f32 per b.
  - Device phase 2: PE-transpose means into meansT[128d, (b,g)] chunks,
    split hi/lo on DVE, then per group g:
    out_g[4b,512o] = mh.T@Wh + mh.T@Wl + ml.T@Wh over 4 d-chunks;
    bias added by the DVE op that also moves PSUM->SBUF.
  - Schedule: ~45 junk warm-up matmuls at kernel start keep the PE's HAM
    clock-gate warm (2.4 GHz) before real work; W streams interleaved
    mid-batch so it neither delays the first tiles nor the phase-2 start.
  - Output per core is [4, G*OUT] = (b, g, o) row-major; host reshapes and
    concatenates over cores.
"""

import ml_dtypes
import numpy as np

import concourse.bacc as bacc
import concourse.mybir as mybir
from concourse import bass_utils
from concourse.masks import make_identity
from concourse.tile import TileContext

B, T, D, G, OUT = 32, 4096, 512, 8, 512
NCORES = 8
BL = B // NCORES  # batch items per core (4)
P = 128
NCH = T // P      # token chunks per batch item (32)
DCH = D // P      # contraction chunks for the projection (4)
QT = 8            # token chunks per batch DMA tile (1 MiB bf16 per DMA)
NQ = NCH // QT
NWARM = 48        # junk matmuls to warm the PE clock gate at start

F32 = mybir.dt.float32
BF16 = mybir.dt.bfloat16
NPBF16 = ml_dtypes.bfloat16

_cache: dict = {}


def _build():
    nc = bacc.Bacc(
        "TRN2", target_bir_lowering=False, debug=False, num_devices=NCORES
    )

    bh_d = nc.dram_tensor("batch_hi", [BL, T, D], BF16, kind="ExternalInput")
    bl_d = nc.dram_tensor("batch_lo", [BL, T, D], BF16, kind="ExternalInput")
    vft_d = nc.dram_tensor("vft", [P, BL * NCH * G], BF16, kind="ExternalInput")
    wh_d = nc.dram_tensor("wt_hi", [P, G * DCH * OUT], BF16, kind="ExternalInput")
    wl_d = nc.dram_tensor("wt_lo", [P, G * DCH * OUT], BF16, kind="ExternalInput")
    bias_d = nc.dram_tensor("biasr", [BL, G * OUT], F32, kind="ExternalInput")
    invc_d = nc.dram_tensor("invc", [G, BL], F32, kind="ExternalInput")
    out_d = nc.dram_tensor("out", [BL, G * OUT], F32, kind="ExternalOutput")

    def dma_batch(pool, dram, b, q, tag):
        bt = pool.tile([P, QT, D], BF16, tag=tag)
        src = dram.ap()[b, q * QT * P:(q + 1) * QT * P, :]
        nc.sync.dma_start(out=bt, in_=src.rearrange("(tc p) d -> p tc d", p=P))
        return bt

    with TileContext(nc) as tc:
        with tc.tile_pool(name="consts", bufs=1) as consts, \
             tc.tile_pool(name="bpool", bufs=5) as bpool, \
             tc.tile_pool(name="pacc", bufs=4, space="PSUM") as pacc, \
             tc.tile_pool(name="ptp", bufs=2, space="PSUM") as ptp, \
             tc.tile_pool(name="pout", bufs=2, space="PSUM") as pout:

            # Small consts first (fast DMAs), then PE warm-up junk matmuls
            # that run while the first batch tiles stream in.
            vf_sb = consts.tile([P, BL * NCH * G], BF16)
            nc.sync.dma_start(out=vf_sb, in_=vft_d.ap())
            bias_sb = consts.tile([BL, G * OUT], F32)
            nc.sync.dma_start(out=bias_sb, in_=bias_d.ap())
            invc_sb = consts.tile([G, BL], F32)
            nc.sync.dma_start(out=invc_sb, in_=invc_d.ap())
            ident = consts.tile([G, G], F32)
            make_identity(nc, ident)
            junk_sb = consts.tile([P, 512], BF16)
            nc.gpsimd.memset(junk_sb, 0.0)
            warm_ps = pacc.tile([G, 512], F32, tag="sums")
            for _ in range(NWARM):
                nc.tensor.matmul(
                    warm_ps, lhsT=junk_sb[:, :G], rhs=junk_sb,
                    start=True, stop=True,
                )

            wh_sb = consts.tile([P, G * DCH * OUT], BF16)
            wl_sb = consts.tile([P, G * DCH * OUT], BF16)

            means_sb = consts.tile([G, BL, D], F32)
            mt_sb = consts.tile([P, DCH, BL * G], F32)
            mth_sb = consts.tile([P, DCH, BL * G], BF16)
            mthf_sb = consts.tile([P, DCH, BL * G], F32)
            mtlf_sb = consts.tile([P, DCH, BL * G], F32)
            mtl_sb = consts.tile([P, DCH, BL * G], BF16)
            out_sb = consts.tile([BL, G, OUT], F32)

            # Phase 1: segment sums, one PSUM bank per local b. W streams
            # interleaved after b0 / b1 so it never gates the pipeline.
            for b in range(BL):
                sums_ps = pacc.tile([G, D], F32, tag="sums")
                for q in range(NQ):
                    bth = dma_batch(bpool, bh_d, b, q, "bth")
                    btl = dma_batch(bpool, bl_d, b, q, "btl")
                    for j in range(QT):
                        c = q * QT + j
                        vfs = vf_sb[:, (b * NCH + c) * G:(b * NCH + c + 1) * G]
                        nc.tensor.matmul(
                            sums_ps, lhsT=vfs, rhs=bth[:, j, :],
                            start=(c == 0), stop=False,
                        )
                        nc.tensor.matmul(
                            sums_ps, lhsT=vfs, rhs=btl[:, j, :],
                            start=False, stop=(c == NCH - 1),
                        )
                if b == 0:
                    nc.sync.dma_start(out=wh_sb, in_=wh_d.ap())
                if b == 1:
                    nc.sync.dma_start(out=wl_sb, in_=wl_d.ap())
                # means_b = sums_b * (1/count_b), [8 g, 512 d] at base 0
                nc.vector.tensor_scalar_mul(
                    means_sb[:, b, :], sums_ps, invc_sb[:, b:b + 1]
                )

            # Transpose means -> mt [128 d, (c, 8b+g)], then split hi/lo.
            for b in range(BL):
                tp = ptp.tile([P, DCH, G], F32, tag="tp")
                for c in range(DCH):
                    nc.tensor.transpose(
                        tp[:, c, :], means_sb[:, b, c * P:(c + 1) * P], ident
                    )
                nc.vector.tensor_copy(
                    out=mt_sb.rearrange("p c (b g) -> p c b g", g=G)[:, :, b, :],
                    in_=tp,
                )
            nc.vector.tensor_copy(out=mth_sb, in_=mt_sb)     # cast to bf16
            nc.vector.tensor_copy(out=mthf_sb, in_=mth_sb)   # back to f32
            nc.vector.tensor_sub(mtlf_sb, mt_sb, mthf_sb)    # residual
            nc.vector.tensor_copy(out=mtl_sb, in_=mtlf_sb)   # cast to bf16

            # Phase 2: per-group projection. lhsT columns {8b+g : b} stride G.
            mh_v = mth_sb.rearrange("p c (b g) -> p c g b", g=G)
            ml_v = mtl_sb.rearrange("p c (b g) -> p c g b", g=G)
            for g in range(G):
                og_ps = pout.tile([BL, OUT], F32, tag="og")
                for c in range(DCH):
                    wh_s = wh_sb[:, (g * DCH + c) * OUT:(g * DCH + c + 1) * OUT]
                    wl_s = wl_sb[:, (g * DCH + c) * OUT:(g * DCH + c + 1) * OUT]
                    nc.tensor.matmul(
                        og_ps, lhsT=mh_v[:, c, g, :], rhs=wh_s,
                        start=(c == 0), stop=False,
                    )
                    nc.tensor.matmul(
                        og_ps, lhsT=mh_v[:, c, g, :], rhs=wl_s,
                        start=False, stop=False,
                    )
                    nc.tensor.matmul(
                        og_ps, lhsT=ml_v[:, c, g, :], rhs=wh_s,
                        start=False, stop=(c == DCH - 1),
                    )
                # bias add + PSUM->SBUF copyback in one op
                nc.vector.tensor_add(
                    out_sb[:, g, :], og_ps, bias_sb[:, g * OUT:(g + 1) * OUT]
                )

            nc.sync.dma_start(
                out=out_d.ap(), in_=out_sb.rearrange("b g o -> b (g o)")
            )

    nc.compile()
    return nc


def _prep(inputs):
    batch = np.asarray(inputs["batch"], dtype=np.float32)
    W = np.asarray(inputs["W"], dtype=np.float32)
    b_bias = np.asarray(inputs["b_bias"], dtype=np.float32)
    tt = np.asarray(inputs["token_types"]).astype(np.int64)
    pad = np.asarray(inputs["key_padding_mask"]).astype(bool)

    batch_hi = batch.astype(NPBF16)
    batch_lo = (batch - batch_hi.astype(np.float32)).astype(NPBF16)

    onehot = tt[:, None] == np.arange(G)[None, :]            # [T, G]
    vf = ((~pad)[:, :, None] & onehot[None, :, :]).astype(np.float32)  # [B,T,G]
    counts = vf.sum(axis=1)                                  # [B, G]
    invc = np.where(counts > 0, 1.0 / np.maximum(counts, 1.0), 0.0).astype(
        np.float32
    )

    # vft[core][p, b*NCH*G + c*G + g] = vf[BL*core + b, c*128 + p, g]
    vft = np.ascontiguousarray(
        vf.reshape(NCORES, BL, NCH, P, G).transpose(0, 3, 1, 2, 4)
    ).reshape(NCORES, P, BL * NCH * G).astype(NPBF16)

    # wt[p, (g*DCH + c)*OUT + o] = W[g, c*128 + p, o], split hi/lo
    wh = W.astype(NPBF16)
    wl = (W - wh.astype(np.float32)).astype(NPBF16)

    def _warr(w):
        return np.ascontiguousarray(
            w.reshape(G, DCH, P, OUT).transpose(2, 0, 1, 3)
        ).reshape(P, G * DCH * OUT)

    wt_hi, wt_lo = _warr(wh), _warr(wl)

    biasr = np.ascontiguousarray(
        np.broadcast_to(b_bias.reshape(1, G * OUT), (BL, G * OUT))
    )
    invc_t = np.ascontiguousarray(
        invc.reshape(NCORES, BL, G).transpose(0, 2, 1)
    )

    in_maps = []
    for c in range(NCORES):
        in_maps.append(
            {
                "batch_hi": np.ascontiguousarray(batch_hi[BL * c:BL * (c + 1)]),
                "batch_lo": np.ascontiguousarray(batch_lo[BL * c:BL * (c + 1)]),
                "vft": vft[c],
                "wt_hi": wt_hi,
                "wt_lo": wt_lo,
                "biasr": biasr,
                "invc": invc_t[c],
            }
        )
    return in_maps


def _gather(results):
    outs = [np.asarray(r["out"]).reshape(BL, G, OUT) for r in results]
    return np.ascontiguousarray(np.concatenate(outs, axis=0))


def kernel(**inputs) -> np.ndarray:
    if "nc" not in _cache:
        _cache["nc"] = _build()
    in_maps = _prep(inputs)
    res = bass_utils.run_bass_kernel_spmd(
        _cache["nc"], in_maps, core_ids=list(range(NCORES))
    )
    return _gather(res.results)
